# revision 14
# baseline (speedup 1.0000x reference)
"""Trainium2 Bass kernel for nn_BernoulliMultiHeadAttention.

Full-input contract: kernel(**inputs) takes the unsharded inputs of
reference.setup_inputs() and returns the full [2, 2048, 1024] f32 output.

Distribution: 8 cores = batch(2) x head-groups(4 heads each). Each core
computes qkv projection for its heads, scores^T = k @ q^T, Bernoulli sampling
via a single compare against a precomputed threshold table
T = 8*logit(U) (U = jax.random.uniform(key(42), [B,H,S,S]) — exactly the bits
jax.random.bernoulli uses), attn^T @ v, and its slice of the out-projection.
Host sums the 4 partial out-projections per batch and adds out_b.

Precision: all matmuls run in fp16 on the PE (1 cyc/col vs 4 for fp32) with
hi/lo error compensation on the q/k path so sampling thresholds keep ~2^-21
relative accuracy. v / attn@v / out-proj are single-pass fp16 (error ~5e-4,
far below sampling noise).
"""

import os
import sys

import numpy as np

for _p in ("/opt/trn_rl_repo", "/root/.axon_site/_ro/trn_rl_repo"):
    if os.path.isdir(_p) and _p not in sys.path:
        sys.path.append(_p)

B, S, E, H, D = 2, 2048, 1024, 16, 64
NCORES = 8
HL = H // (NCORES // B)  # heads per core = 4
P = 128

_STATE = {}


# ----------------------------------------------------------------------------
# device program
# ----------------------------------------------------------------------------

def build_program(s=S, e=E, hl=HL, d=D, debug=False, repeat=0):
    """repeat=0: plain program (graded path). repeat=N>0: wrap the whole body
    in a hardware For_i loop running it N times — used only for timing
    measurement (delta vs the plain program divides out tunnel overhead)."""
    import concourse.mybir as mybir
    import concourse.tile as tile
    from concourse import bacc
    from concourse.alu_op_type import AluOpType
    from contextlib import ExitStack, nullcontext

    f32 = mybir.dt.float32
    f16 = mybir.dt.float16
    Copy = mybir.ActivationFunctionType.Copy

    assert d == 64 and P % d == 0
    NQK = 2 * hl * d          # q+k projection rows
    NV = hl * d               # v rows
    MQK = NQK // P            # qk psum M-tiles
    NEC = e // P              # contraction chunks for projections
    NB = min(512, s)          # matmul moving block
    NQB = s // NB
    NBE = min(512, e)
    SC_N = NB                 # scores psum tile width (1 bank)
    NSC = s // SC_N
    NT = s // P               # key tiles / token tiles
    NDC = max(1, NV // P)     # out-proj contraction chunks

    nc = bacc.Bacc(None, target_bir_lowering=False, debug=debug)

    t_d = nc.dram_tensor("t", [hl, s, s], f32, kind="ExternalInput")
    xhi_d = nc.dram_tensor("xhi", [e, s], f16, kind="ExternalInput")
    xlo_d = nc.dram_tensor("xlo", [e, s], f16, kind="ExternalInput")
    wqkhi_d = nc.dram_tensor("wqkhi", [e, NQK], f16, kind="ExternalInput")
    wqklo_d = nc.dram_tensor("wqklo", [e, NQK], f16, kind="ExternalInput")
    wvhi_d = nc.dram_tensor("wvhi", [e, NV], f16, kind="ExternalInput")
    wohi_d = nc.dram_tensor("wohi", [NV, e], f16, kind="ExternalInput")
    bqk_d = nc.dram_tensor("bqk", [1, NQK], f16, kind="ExternalInput")
    bv_d = nc.dram_tensor("bv", [1, NV], f16, kind="ExternalInput")
    out_d = nc.dram_tensor("out", [s, e], f32, kind="ExternalOutput")

    with tile.TileContext(nc) as tc, ExitStack() as ctx:
        const = ctx.enter_context(tc.tile_pool(name="const", bufs=1))
        pp = ctx.enter_context(tc.tile_pool(name="pp", bufs=4, space="PSUM"))
        avp = ctx.enter_context(tc.tile_pool(name="avp", bufs=1, space="PSUM"))
        tp = ctx.enter_context(tc.tile_pool(name="tp", bufs=3))
        sp = ctx.enter_context(tc.tile_pool(name="sp", bufs=8))
        op = ctx.enter_context(tc.tile_pool(name="op", bufs=3))

        loop_cm = tc.For_i(0, repeat, 1) if repeat else nullcontext()
        ctx.enter_context(loop_cm)

        # --- tiles ---
        x_hi = const.tile([P, NEC, s], f16, tag="x_hi")
        x_lo = const.tile([P, NEC, s], f16, tag="x_lo")
        wqk_hi = const.tile([P, NEC, NQK], f16, tag="wqk_hi")
        wqk_lo = const.tile([P, NEC, NQK], f16, tag="wqk_lo")
        wv_hi = const.tile([P, NEC, NV], f16, tag="wv_hi")
        wo_sb = const.tile([P, NDC, e], f16, tag="wo_sb")
        bqk_sb = const.tile([1, NQK], f16, tag="bqk_sb")
        bv_sb = const.tile([1, NV], f16, tag="bv_sb")
        ones = const.tile([1, NB], f16, tag="ones")
        # per-head stores: q duplicated hi/lo ([qhi;qhi] on partitions),
        # k stacked hi over lo ([khi;klo]) for the 2-pass exact product.
        qhi = [const.tile([2 * d, s], f16, tag=f"qhi{h}", name=f"qhi{h}") for h in range(hl)]
        qlo = [const.tile([2 * d, s], f16, tag=f"qlo{h}", name=f"qlo{h}") for h in range(hl)]
        kst = [const.tile([2 * d, s], f16, tag=f"kst{h}", name=f"kst{h}") for h in range(hl)]
        v_sb = const.tile([P, NT, NV], f16, tag="v_sb")
        avT = const.tile([P, NDC, s], f16, tag="avT")

        # --- loads in consumption order (v path first: smallest working set) ---
        nc.vector.memset(ones, 1.0)
        nc.sync.dma_start(x_hi, xhi_d.rearrange("(c p) s -> p c s", p=P))
        nc.sync.dma_start(wv_hi, wvhi_d.rearrange("(c p) n -> p c n", p=P))
        nc.sync.dma_start(bv_sb, bv_d[:, :])
        nc.sync.dma_start(wqk_hi, wqkhi_d.rearrange("(c p) n -> p c n", p=P))
        nc.sync.dma_start(wqk_lo, wqklo_d.rearrange("(c p) n -> p c n", p=P))
        nc.sync.dma_start(x_lo, xlo_d.rearrange("(c p) s -> p c s", p=P))
        nc.sync.dma_start(bqk_sb, bqk_d[:, :])
        nc.sync.dma_start(wo_sb, wohi_d.rearrange("(c p) n -> p c n", p=P))

        # --- phase 1a: v projection (1-pass fp16) ---
        for ti in range(NT):
            ps = pp.tile([P, NV], f32, tag="mm")
            nc.tensor.matmul(
                ps, ones[0:1, 0:P], bv_sb[0:1, :], start=True, stop=False
            )
            for ec in range(NEC):
                nc.tensor.matmul(
                    ps,
                    x_hi[:, ec, ti * P:(ti + 1) * P],
                    wv_hi[:, ec, :],
                    start=False,
                    stop=(ec == NEC - 1),
                )
            nc.scalar.activation(v_sb[:, ti, :], ps, Copy)

        # --- phase 1b: q/k projection (3-pass fp16 hi/lo) ---
        passes = [(wqk_hi, x_hi), (wqk_hi, x_lo), (wqk_lo, x_hi)]

        def emit_qk(mi):
            for nb in range(NQB):
                ps = pp.tile([P, NB], f32, tag="mm")
                nc.tensor.matmul(
                    ps, bqk_sb[0:1, mi * P:(mi + 1) * P], ones[0:1, 0:NB],
                    start=True, stop=False,
                )
                for ec in range(NEC):
                    for pi, (wt, xt) in enumerate(passes):
                        nc.tensor.matmul(
                            ps,
                            wt[:, ec, mi * P:(mi + 1) * P],
                            xt[:, ec, nb * NB:(nb + 1) * NB],
                            start=False,
                            stop=(ec == NEC - 1 and pi == len(passes) - 1),
                        )
                tsl = slice(nb * NB, (nb + 1) * NB)
                for sub in range(P // d):
                    r = mi * P + sub * d
                    src = ps[sub * d:(sub + 1) * d, :]
                    if r < hl * d:
                        h = r // d
                        nc.scalar.activation(qhi[h][0:d, tsl], src, Copy)
                        nc.scalar.activation(qhi[h][d:2 * d, tsl], src, Copy)
                        nc.vector.tensor_tensor(
                            out=qlo[h][0:d, tsl], in0=src,
                            in1=qhi[h][0:d, tsl], op=AluOpType.subtract,
                        )
                        nc.scalar.activation(
                            qlo[h][d:2 * d, tsl], qlo[h][0:d, tsl], Copy
                        )
                    else:
                        h = (r - hl * d) // d
                        nc.scalar.activation(kst[h][0:d, tsl], src, Copy)
                        nc.vector.tensor_tensor(
                            out=kst[h][d:2 * d, tsl], in0=src,
                            in1=kst[h][0:d, tsl], op=AluOpType.subtract,
                        )

        # --- phase 2: per head: scores^T -> sample -> attn^T @ v ---
        def emit_head(h):
            av = avp.tile([d, s], mybir.dt.float32, tag="av", name=f"av{h}")

            def emit_av(kt, sams):
                for qb, sm in sams:
                    qc = qb * SC_N
                    nc.tensor.matmul(
                        av[0:d, qc:qc + SC_N],
                        v_sb[:, kt, h * d:(h + 1) * d],
                        sm[:, :],
                        start=(kt == 0),
                        stop=(kt == NT - 1),
                    )

            pend = None
            for kt in range(NT):
                tt = tp.tile([P, s], mybir.dt.float32, tag="tt")
                nc.sync.dma_start(tt, t_d[h, kt * P:(kt + 1) * P, :])
                kl = kst[h][:, kt * P:(kt + 1) * P]
                cur = []
                for qb in range(NSC):
                    sc = pp.tile([P, SC_N], mybir.dt.float32, tag="mm")
                    qc = qb * SC_N
                    nc.tensor.matmul(
                        sc, kl, qhi[h][:, qc:qc + SC_N], start=True, stop=False
                    )
                    nc.tensor.matmul(
                        sc, kl, qlo[h][:, qc:qc + SC_N], start=False, stop=True
                    )
                    cur.append((qb, sc))
                if pend is not None:
                    emit_av(*pend)
                sams = []
                for qb, sc in cur:
                    sm = sp.tile([P, SC_N], f16, tag="sm")
                    nc.vector.tensor_tensor(
                        out=sm, in0=tt[:, qb * SC_N:(qb + 1) * SC_N], in1=sc,
                        op=AluOpType.is_lt,
                    )
                    sams.append((qb, sm))
                pend = (kt, sams)
            emit_av(*pend)

            dc = (h * d) // P
            r0 = (h * d) % P
            nc.scalar.activation(avT[r0:r0 + d, dc, :], av[0:d, :], Copy)

        # interleave: head h only needs q tile (h*d)//P and k tile
        # MQK//2 + (h*d)//P; emitting head 0 right after its two qk tiles
        # lets the T stream start ~60us earlier.
        NQT = MQK // 2
        emit_qk(0)
        emit_qk(NQT)
        emit_head(0)
        for mi in range(1, NQT):
            emit_qk(mi)
            emit_qk(NQT + mi)
        for h in range(1, hl):
            emit_head(h)

        # --- phase 3: out projection (partial; host sums across head groups) ---
        NEB = e // NBE
        for ti in range(NT):
            ob = op.tile([P, NEB, NBE], mybir.dt.float32, tag="ob")
            for eb in range(NEB):
                po = pp.tile([P, NBE], mybir.dt.float32, tag="mm")
                for dcc in range(NDC):
                    nc.tensor.matmul(
                        po,
                        avT[:, dcc, ti * P:(ti + 1) * P],
                        wo_sb[:, dcc, eb * NBE:(eb + 1) * NBE],
                        start=(dcc == 0),
                        stop=(dcc == NDC - 1),
                    )
                nc.scalar.activation(ob[:, eb, :], po, Copy)
            nc.sync.dma_start(
                out_d.rearrange("s (c n) -> s c n", n=NBE)[
                    ti * P:(ti + 1) * P, :, :
                ],
                ob,
            )

    nc.finalize()
    return nc


# ----------------------------------------------------------------------------
# host side
# ----------------------------------------------------------------------------

def _build_t_shards():
    """T[c, h_local, kk, q] = 8*logit(U[b, h, q, kk]) for core c=(b, h_grp).

    U must be bit-identical to what jax.random.bernoulli(key(42), probs)
    compares against, i.e. jax.random.uniform(key(42), (B,H,S,S), f32) under
    this environment's PRNG impl/backend. Uniform+transpose run through jax;
    logit runs on host in f32 (monotone map, ~1ulp, flips are negligible).
    """
    cache = "/root/problem/t_shards_v2.npy"
    if os.path.exists(cache):
        try:
            t = np.load(cache, mmap_mode=None)
            if t.shape == (NCORES, HL, S, S):
                return np.ascontiguousarray(t)
        except Exception:
            pass

    import jax
    import jax.numpy as jnp

    # IMPORTANT: this exact standalone call produces the same bits as the
    # uniform inside jax.random.bernoulli's compiled program (verified
    # 0/134M mismatch). rbg bits are compilation-dependent, so do NOT fuse
    # anything else (transpose/logit) into this computation.
    u = np.asarray(
        jax.random.uniform(jax.random.key(42), (B, H, S, S), jnp.float32)
    )

    t = np.empty((NCORES, HL, S, S), np.float32)
    one = np.float32(1.0)
    eight = np.float32(8.0)
    gpb = NCORES // B
    with np.errstate(divide="ignore"):
        for b in range(B):
            for h in range(H):
                blk = np.ascontiguousarray(u[b, h].T)  # [kk, q]
                np.divide(blk, one - blk, out=blk)
                np.log(blk, out=blk)
                np.multiply(blk, eight, out=blk)
                t[b * gpb + h // HL, h % HL] = blk
    return t


def _split16(a):
    hi = a.astype(np.float16)
    lo = (a - hi.astype(np.float32)).astype(np.float16)
    return np.ascontiguousarray(hi), np.ascontiguousarray(lo)


def _prep_in_maps(x, qkv_w, qkv_b, out_w, t_shards):
    x = np.asarray(x, np.float32)
    qkv_w = np.asarray(qkv_w, np.float32)
    qkv_b = np.asarray(qkv_b, np.float32)
    out_w = np.asarray(out_w, np.float32)

    xs = []
    for b in range(B):
        xT = np.ascontiguousarray(x[b].T)  # [E, S]
        xs.append(_split16(xT))

    in_maps = []
    for c in range(NCORES):
        b = c // (NCORES // B)
        g = c % (NCORES // B)
        r0 = g * HL * D
        r1 = r0 + HL * D
        wqk = np.concatenate([qkv_w[r0:r1], qkv_w[E + r0:E + r1]], axis=0)
        wqk_hi, wqk_lo = _split16(wqk.T)  # [E, 2*HL*D]
        wv_hi = np.ascontiguousarray(qkv_w[2 * E + r0:2 * E + r1].T).astype(np.float16)
        wo_hi = np.ascontiguousarray(out_w[:, r0:r1].T).astype(np.float16)
        bqk = np.concatenate([qkv_b[r0:r1], qkv_b[E + r0:E + r1]])
        bv = qkv_b[2 * E + r0:2 * E + r1]
        xhi, xlo = xs[b]
        in_maps.append({
            "t": t_shards[c],
            "xhi": xhi, "xlo": xlo,
            "wqkhi": wqk_hi, "wqklo": wqk_lo,
            "wvhi": wv_hi, "wohi": wo_hi,
            "bqk": bqk.astype(np.float16).reshape(1, -1),
            "bv": bv.astype(np.float16).reshape(1, -1),
        })
    return in_maps


def kernel(**inputs):
    from concourse.bass_utils import run_bass_kernel_spmd

    x = np.asarray(inputs["x"], np.float32)
    qkv_w = np.asarray(inputs["qkv_w"], np.float32)
    qkv_b = np.asarray(inputs["qkv_b"], np.float32)
    out_w = np.asarray(inputs["out_w"], np.float32)
    out_b = np.asarray(inputs["out_b"], np.float32)

    if "t" not in _STATE:
        _STATE["t"] = _build_t_shards()
    if "nc" not in _STATE:
        _STATE["nc"] = build_program()

    in_maps = _prep_in_maps(x, qkv_w, qkv_b, out_w, _STATE["t"])
    kw = _STATE.get("run_kwargs", {})
    res = run_bass_kernel_spmd(_STATE["nc"], in_maps, list(range(NCORES)), **kw)
    _STATE["last_results"] = res

    gpb = NCORES // B
    out = np.empty((B, S, E), np.float32)
    for b in range(B):
        acc = res.results[b * gpb]["out"].astype(np.float32)
        for g in range(1, gpb):
            acc = acc + res.results[b * gpb + g]["out"]
        out[b] = acc + out_b[None, :]
    return out


# revision 21
# speedup vs baseline: 1.0400x; 1.0400x over previous
"""Trainium2 Bass kernel for nn_BernoulliMultiHeadAttention.

Full-input contract: kernel(**inputs) takes the unsharded inputs of
reference.setup_inputs() and returns the full [2, 2048, 1024] f32 output.

Distribution: 8 cores = batch(2) x head-groups(4 heads each). Each core
computes qkv projection for its heads, scores^T = k @ q^T, Bernoulli sampling
via a single compare against a precomputed threshold table
T = 8*logit(U) (U = jax.random.uniform(key(42), [B,H,S,S]) — exactly the bits
jax.random.bernoulli uses), attn^T @ v, and its slice of the out-projection.
Host sums the 4 partial out-projections per batch and adds out_b.

Precision: all matmuls run in fp16 on the PE (1 cyc/col vs 4 for fp32) with
hi/lo error compensation on the q/k path so sampling thresholds keep ~2^-21
relative accuracy. v / attn@v / out-proj are single-pass fp16 (error ~5e-4,
far below sampling noise).
"""

import os
import sys

import numpy as np

for _p in ("/opt/trn_rl_repo", "/root/.axon_site/_ro/trn_rl_repo"):
    if os.path.isdir(_p) and _p not in sys.path:
        sys.path.append(_p)

B, S, E, H, D = 2, 2048, 1024, 16, 64
NCORES = 8
HL = H // (NCORES // B)  # heads per core = 4
P = 128

_STATE = {}


# ----------------------------------------------------------------------------
# device program
# ----------------------------------------------------------------------------

def build_program(s=S, e=E, hl=HL, d=D, debug=False, repeat=0,
                  skip_tdma=False, skip_cmpav=False, skip_lo=False, salt=""):
    """repeat=0: plain program (graded path). repeat=N>0: wrap the whole body
    in a hardware For_i loop running it N times — used only for timing
    measurement (delta vs the plain program divides out tunnel overhead).
    skip_* flags build timing-bisection variants (not numerically valid)."""
    import concourse.mybir as mybir
    import concourse.tile as tile
    from concourse import bacc
    from concourse.alu_op_type import AluOpType
    from contextlib import ExitStack, nullcontext

    f32 = mybir.dt.float32
    f16 = mybir.dt.float16
    Copy = mybir.ActivationFunctionType.Copy

    assert d == 64 and P % d == 0
    NQK = 2 * hl * d          # q+k projection rows
    NV = hl * d               # v rows
    MQK = NQK // P            # qk psum M-tiles
    NEC = e // P              # contraction chunks for projections
    NB = min(512, s)          # matmul moving block
    NQB = s // NB
    NBE = min(512, e)
    SC_N = NB                 # scores psum tile width (1 bank)
    NSC = s // SC_N
    NT = s // P               # key tiles / token tiles
    NDC = max(1, NV // P)     # out-proj contraction chunks

    nc = bacc.Bacc(None, target_bir_lowering=False, debug=debug)

    t_d = nc.dram_tensor("t", [hl, s, s], f32, kind="ExternalInput")
    xhi_d = nc.dram_tensor("xhi", [e, s], f16, kind="ExternalInput")
    xlo_d = nc.dram_tensor("xlo", [e, s], f16, kind="ExternalInput")
    wqkhi_d = nc.dram_tensor("wqkhi", [e, NQK], f16, kind="ExternalInput")
    wqklo_d = nc.dram_tensor("wqklo", [e, NQK], f16, kind="ExternalInput")
    wvhi_d = nc.dram_tensor("wvhi", [e, NV], f16, kind="ExternalInput")
    wohi_d = nc.dram_tensor("wohi", [NV, e], f16, kind="ExternalInput")
    bqk_d = nc.dram_tensor("bqk", [1, NQK], f16, kind="ExternalInput")
    bv_d = nc.dram_tensor("bv", [1, NV], f16, kind="ExternalInput")
    out_d = nc.dram_tensor("out", [s, e], f32, kind="ExternalOutput")
    if salt:
        nc.dram_tensor(f"salt_{salt}", [1, 1], f32)  # cache-bust marker

    with tile.TileContext(nc) as tc, ExitStack() as ctx:
        const = ctx.enter_context(tc.tile_pool(name="const", bufs=1))
        pp = ctx.enter_context(tc.tile_pool(name="pp", bufs=4, space="PSUM"))
        avp = ctx.enter_context(tc.tile_pool(name="avp", bufs=1, space="PSUM"))
        tp = ctx.enter_context(tc.tile_pool(name="tp", bufs=3))
        sp = ctx.enter_context(tc.tile_pool(name="sp", bufs=8))
        op = ctx.enter_context(tc.tile_pool(name="op", bufs=3))

        loop_cm = tc.For_i(0, repeat, 1) if repeat else nullcontext()
        ctx.enter_context(loop_cm)

        # --- tiles ---
        x_hi = const.tile([P, NEC, s], f16, tag="x_hi")
        x_lo = const.tile([P, NEC, s], f16, tag="x_lo")
        wqk_hi = const.tile([P, NEC, NQK], f16, tag="wqk_hi")
        wqk_lo = const.tile([P, NEC, NQK], f16, tag="wqk_lo")
        wv_hi = const.tile([P, NEC, NV], f16, tag="wv_hi")
        wo_sb = const.tile([P, NDC, e], f16, tag="wo_sb")
        bqk_sb = const.tile([1, NQK], f16, tag="bqk_sb")
        bv_sb = const.tile([1, NV], f16, tag="bv_sb")
        ones = const.tile([1, NB], f16, tag="ones")
        # per-head stores: q duplicated hi/lo ([qhi;qhi] on partitions),
        # k stacked hi over lo ([khi;klo]) for the 2-pass exact product.
        qhi = [const.tile([2 * d, s], f16, tag=f"qhi{h}", name=f"qhi{h}") for h in range(hl)]
        qlo = [const.tile([2 * d, s], f16, tag=f"qlo{h}", name=f"qlo{h}") for h in range(hl)]
        kst = [const.tile([2 * d, s], f16, tag=f"kst{h}", name=f"kst{h}") for h in range(hl)]
        v_sb = const.tile([P, NT, NV], f16, tag="v_sb")
        avT = const.tile([P, NDC, s], f16, tag="avT")

        # --- loads in consumption order (v path first: smallest working set) ---
        nc.vector.memset(ones, 1.0)
        nc.sync.dma_start(x_hi, xhi_d.rearrange("(c p) s -> p c s", p=P))
        nc.sync.dma_start(wv_hi, wvhi_d.rearrange("(c p) n -> p c n", p=P))
        nc.sync.dma_start(bv_sb, bv_d[:, :])
        nc.sync.dma_start(wqk_hi, wqkhi_d.rearrange("(c p) n -> p c n", p=P))
        nc.sync.dma_start(wqk_lo, wqklo_d.rearrange("(c p) n -> p c n", p=P))
        nc.sync.dma_start(x_lo, xlo_d.rearrange("(c p) s -> p c s", p=P))
        nc.sync.dma_start(bqk_sb, bqk_d[:, :])
        nc.sync.dma_start(wo_sb, wohi_d.rearrange("(c p) n -> p c n", p=P))

        # --- phase 1a: v projection (1-pass fp16) ---
        for ti in range(NT):
            ps = pp.tile([P, NV], f32, tag="mm")
            nc.tensor.matmul(
                ps, ones[0:1, 0:P], bv_sb[0:1, :], start=True, stop=False
            )
            for ec in range(NEC):
                nc.tensor.matmul(
                    ps,
                    x_hi[:, ec, ti * P:(ti + 1) * P],
                    wv_hi[:, ec, :],
                    start=False,
                    stop=(ec == NEC - 1),
                )
            nc.scalar.activation(v_sb[:, ti, :], ps, Copy)

        # --- phase 1b: q/k projection (3-pass fp16 hi/lo) ---
        passes = [(wqk_hi, x_hi), (wqk_hi, x_lo), (wqk_lo, x_hi)]

        def emit_qk(mi):
            for nb in range(NQB):
                ps = pp.tile([P, NB], f32, tag="mm")
                nc.tensor.matmul(
                    ps, bqk_sb[0:1, mi * P:(mi + 1) * P], ones[0:1, 0:NB],
                    start=True, stop=False,
                )
                for ec in range(NEC):
                    for pi, (wt, xt) in enumerate(passes):
                        nc.tensor.matmul(
                            ps,
                            wt[:, ec, mi * P:(mi + 1) * P],
                            xt[:, ec, nb * NB:(nb + 1) * NB],
                            start=False,
                            stop=(ec == NEC - 1 and pi == len(passes) - 1),
                        )
                tsl = slice(nb * NB, (nb + 1) * NB)
                for sub in range(P // d):
                    r = mi * P + sub * d
                    src = ps[sub * d:(sub + 1) * d, :]
                    if r < hl * d:
                        h = r // d
                        nc.scalar.activation(qhi[h][0:d, tsl], src, Copy)
                        nc.scalar.activation(qhi[h][d:2 * d, tsl], src, Copy)
                        nc.vector.tensor_tensor(
                            out=qlo[h][0:d, tsl], in0=src,
                            in1=qhi[h][0:d, tsl], op=AluOpType.subtract,
                        )
                        nc.scalar.activation(
                            qlo[h][d:2 * d, tsl], qlo[h][0:d, tsl], Copy
                        )
                    else:
                        h = (r - hl * d) // d
                        nc.scalar.activation(kst[h][0:d, tsl], src, Copy)
                        nc.vector.tensor_tensor(
                            out=kst[h][d:2 * d, tsl], in0=src,
                            in1=kst[h][0:d, tsl], op=AluOpType.subtract,
                        )

        # --- phase 2: per head: scores^T -> sample -> attn^T @ v ---
        tdummy = samdummy = None
        if skip_tdma:
            tdummy = const.tile([P, s], f32, tag="tdummy")
            nc.vector.memset(tdummy, 0.5)
        if skip_cmpav:
            samdummy = const.tile([P, s], f16, tag="samdummy")
            nc.vector.memset(avT, 0.0)

        def emit_head(h):
            av = None
            if not skip_cmpav:
                av = avp.tile([d, s], mybir.dt.float32, tag="av", name=f"av{h}")

            def emit_av(kt, sams):
                for qb, sm in sams:
                    qc = qb * SC_N
                    nc.tensor.matmul(
                        av[0:d, qc:qc + SC_N],
                        v_sb[:, kt, h * d:(h + 1) * d],
                        sm[:, :],
                        start=(kt == 0),
                        stop=(kt == NT - 1),
                    )

            pend = None
            for kt in range(NT):
                if not skip_tdma:
                    tt = tp.tile([P, s], mybir.dt.float32, tag="tt")
                    nc.sync.dma_start(tt, t_d[h, kt * P:(kt + 1) * P, :])
                else:
                    tt = tdummy
                kl = kst[h][:, kt * P:(kt + 1) * P]
                cur = []
                for qb in range(NSC):
                    sc = pp.tile([P, SC_N], mybir.dt.float32, tag="mm")
                    qc = qb * SC_N
                    nc.tensor.matmul(
                        sc, kl, qhi[h][:, qc:qc + SC_N], start=True,
                        stop=skip_lo,
                    )
                    if not skip_lo:
                        nc.tensor.matmul(
                            sc, kl, qlo[h][:, qc:qc + SC_N], start=False, stop=True
                        )
                    cur.append((qb, sc))
                if pend is not None:
                    emit_av(*pend)
                sams = []
                for qb, sc in cur:
                    if skip_cmpav:
                        # still release sc via a reader so psum slots cycle
                        nc.scalar.activation(samdummy[:, 0:SC_N], sc, Copy)
                        continue
                    sm = sp.tile([P, SC_N], f16, tag="sm")
                    nc.vector.tensor_tensor(
                        out=sm, in0=tt[:, qb * SC_N:(qb + 1) * SC_N], in1=sc,
                        op=AluOpType.is_lt,
                    )
                    sams.append((qb, sm))
                pend = (kt, sams) if sams else None
            if pend is not None:
                emit_av(*pend)

            if not skip_cmpav:
                dc = (h * d) // P
                r0 = (h * d) % P
                nc.scalar.activation(avT[r0:r0 + d, dc, :], av[0:d, :], Copy)

        # interleave: head h only needs q tile (h*d)//P and k tile
        # MQK//2 + (h*d)//P; emitting head 0 right after its two qk tiles
        # lets the T stream start ~60us earlier.
        NQT = MQK // 2
        emit_qk(0)
        emit_qk(NQT)
        emit_head(0)
        for mi in range(1, NQT):
            emit_qk(mi)
            emit_qk(NQT + mi)
        for h in range(1, hl):
            emit_head(h)

        # --- phase 3: out projection (partial; host sums across head groups) ---
        NEB = e // NBE
        for ti in range(NT):
            ob = op.tile([P, NEB, NBE], mybir.dt.float32, tag="ob")
            for eb in range(NEB):
                po = pp.tile([P, NBE], mybir.dt.float32, tag="mm")
                for dcc in range(NDC):
                    nc.tensor.matmul(
                        po,
                        avT[:, dcc, ti * P:(ti + 1) * P],
                        wo_sb[:, dcc, eb * NBE:(eb + 1) * NBE],
                        start=(dcc == 0),
                        stop=(dcc == NDC - 1),
                    )
                nc.scalar.activation(ob[:, eb, :], po, Copy)
            nc.sync.dma_start(
                out_d.rearrange("s (c n) -> s c n", n=NBE)[
                    ti * P:(ti + 1) * P, :, :
                ],
                ob,
            )

    nc.finalize()
    return nc


# ----------------------------------------------------------------------------
# host side
# ----------------------------------------------------------------------------

def _build_t_shards():
    """T[c, h_local, kk, q] = 8*logit(U[b, h, q, kk]) for core c=(b, h_grp).

    U must be bit-identical to what jax.random.bernoulli(key(42), probs)
    compares against, i.e. jax.random.uniform(key(42), (B,H,S,S), f32) under
    this environment's PRNG impl/backend. Uniform+transpose run through jax;
    logit runs on host in f32 (monotone map, ~1ulp, flips are negligible).
    """
    cache = "/root/problem/t_shards_v2.npy"
    if os.path.exists(cache):
        try:
            t = np.load(cache, mmap_mode=None)
            if t.shape == (NCORES, HL, S, S):
                return np.ascontiguousarray(t)
        except Exception:
            pass

    import jax
    import jax.numpy as jnp

    # IMPORTANT: this exact standalone call produces the same bits as the
    # uniform inside jax.random.bernoulli's compiled program (verified
    # 0/134M mismatch). rbg bits are compilation-dependent, so do NOT fuse
    # anything else (transpose/logit) into this computation.
    u = np.asarray(
        jax.random.uniform(jax.random.key(42), (B, H, S, S), jnp.float32)
    )

    t = np.empty((NCORES, HL, S, S), np.float32)
    one = np.float32(1.0)
    eight = np.float32(8.0)
    gpb = NCORES // B
    with np.errstate(divide="ignore"):
        for b in range(B):
            for h in range(H):
                blk = np.ascontiguousarray(u[b, h].T)  # [kk, q]
                np.divide(blk, one - blk, out=blk)
                np.log(blk, out=blk)
                np.multiply(blk, eight, out=blk)
                t[b * gpb + h // HL, h % HL] = blk
    return t


def _split16(a):
    hi = a.astype(np.float16)
    lo = (a - hi.astype(np.float32)).astype(np.float16)
    return np.ascontiguousarray(hi), np.ascontiguousarray(lo)


def _prep_in_maps(x, qkv_w, qkv_b, out_w, t_shards):
    x = np.asarray(x, np.float32)
    qkv_w = np.asarray(qkv_w, np.float32)
    qkv_b = np.asarray(qkv_b, np.float32)
    out_w = np.asarray(out_w, np.float32)

    xs = []
    for b in range(B):
        xT = np.ascontiguousarray(x[b].T)  # [E, S]
        xs.append(_split16(xT))

    in_maps = []
    for c in range(NCORES):
        b = c // (NCORES // B)
        g = c % (NCORES // B)
        r0 = g * HL * D
        r1 = r0 + HL * D
        wqk = np.concatenate([qkv_w[r0:r1], qkv_w[E + r0:E + r1]], axis=0)
        wqk_hi, wqk_lo = _split16(wqk.T)  # [E, 2*HL*D]
        wv_hi = np.ascontiguousarray(qkv_w[2 * E + r0:2 * E + r1].T).astype(np.float16)
        wo_hi = np.ascontiguousarray(out_w[:, r0:r1].T).astype(np.float16)
        bqk = np.concatenate([qkv_b[r0:r1], qkv_b[E + r0:E + r1]])
        bv = qkv_b[2 * E + r0:2 * E + r1]
        xhi, xlo = xs[b]
        in_maps.append({
            "t": t_shards[c],
            "xhi": xhi, "xlo": xlo,
            "wqkhi": wqk_hi, "wqklo": wqk_lo,
            "wvhi": wv_hi, "wohi": wo_hi,
            "bqk": bqk.astype(np.float16).reshape(1, -1),
            "bv": bv.astype(np.float16).reshape(1, -1),
        })
    return in_maps


def kernel(**inputs):
    from concourse.bass_utils import run_bass_kernel_spmd

    x = np.asarray(inputs["x"], np.float32)
    qkv_w = np.asarray(inputs["qkv_w"], np.float32)
    qkv_b = np.asarray(inputs["qkv_b"], np.float32)
    out_w = np.asarray(inputs["out_w"], np.float32)
    out_b = np.asarray(inputs["out_b"], np.float32)

    if "t" not in _STATE:
        _STATE["t"] = _build_t_shards()
    if "nc" not in _STATE:
        _STATE["nc"] = build_program()

    in_maps = _prep_in_maps(x, qkv_w, qkv_b, out_w, _STATE["t"])
    kw = _STATE.get("run_kwargs", {})
    res = run_bass_kernel_spmd(_STATE["nc"], in_maps, list(range(NCORES)), **kw)
    _STATE["last_results"] = res

    gpb = NCORES // B
    out = np.empty((B, S, E), np.float32)
    for b in range(B):
        acc = res.results[b * gpb]["out"].astype(np.float32)
        for g in range(1, gpb):
            acc = acc + res.results[b * gpb + g]["out"]
        out[b] = acc + out_b[None, :]
    return out


# revision 22
# speedup vs baseline: 1.0451x; 1.0049x over previous
"""Trainium2 Bass kernel for nn_BernoulliMultiHeadAttention.

Full-input contract: kernel(**inputs) takes the unsharded inputs of
reference.setup_inputs() and returns the full [2, 2048, 1024] f32 output.

Distribution: 8 cores = batch(2) x head-groups(4 heads each). Each core
computes qkv projection for its heads, scores^T = k @ q^T, Bernoulli sampling
via a single compare against a precomputed threshold table
T = 8*logit(U) (U = jax.random.uniform(key(42), [B,H,S,S]) — exactly the bits
jax.random.bernoulli uses), attn^T @ v, and its slice of the out-projection.
Host sums the 4 partial out-projections per batch and adds out_b.

Precision: all matmuls run in fp16 on the PE (1 cyc/col vs 4 for fp32) with
hi/lo error compensation on the q/k path so sampling thresholds keep ~2^-21
relative accuracy. v / attn@v / out-proj are single-pass fp16 (error ~5e-4,
far below sampling noise).
"""

import os
import sys

import numpy as np

for _p in ("/opt/trn_rl_repo", "/root/.axon_site/_ro/trn_rl_repo"):
    if os.path.isdir(_p) and _p not in sys.path:
        sys.path.append(_p)

B, S, E, H, D = 2, 2048, 1024, 16, 64
NCORES = 8
HL = H // (NCORES // B)  # heads per core = 4
P = 128

_STATE = {}


# ----------------------------------------------------------------------------
# device program
# ----------------------------------------------------------------------------

def build_program(s=S, e=E, hl=HL, d=D, debug=False, repeat=0,
                  skip_tdma=False, skip_cmpav=False, skip_lo=False, salt="",
                  lag=2, tp_bufs=4, scb_bufs=8, sm_bufs=16):
    """repeat=0: plain program (graded path). repeat=N>0: wrap the whole body
    in a hardware For_i loop running it N times — used only for timing
    measurement (delta vs the plain program divides out tunnel overhead).
    skip_* flags build timing-bisection variants (not numerically valid)."""
    import concourse.mybir as mybir
    import concourse.tile as tile
    from concourse import bacc
    from concourse.alu_op_type import AluOpType
    from contextlib import ExitStack, nullcontext
    from collections import deque

    f32 = mybir.dt.float32
    f16 = mybir.dt.float16
    f8 = mybir.dt.float8e4
    Copy = mybir.ActivationFunctionType.Copy

    assert d == 64 and P % d == 0
    NQK = 2 * hl * d          # q+k projection rows
    NV = hl * d               # v rows
    MQK = NQK // P            # qk psum M-tiles
    NEC = e // P              # contraction chunks for projections
    NB = min(512, s)          # matmul moving block
    NQB = s // NB
    NBE = min(512, e)
    SC_N = NB                 # scores psum tile width (1 bank)
    NSC = s // SC_N
    NT = s // P               # key tiles / token tiles
    NDC = max(1, NV // P)     # out-proj contraction chunks

    nc = bacc.Bacc(None, target_bir_lowering=False, debug=debug)

    t_d = nc.dram_tensor("t", [hl, s, s], f32, kind="ExternalInput")
    xhi_d = nc.dram_tensor("xhi", [e, s], f16, kind="ExternalInput")
    xlo_d = nc.dram_tensor("xlo", [e, s], f16, kind="ExternalInput")
    wqkhi_d = nc.dram_tensor("wqkhi", [e, NQK], f16, kind="ExternalInput")
    wqklo_d = nc.dram_tensor("wqklo", [e, NQK], f16, kind="ExternalInput")
    wvhi_d = nc.dram_tensor("wvhi", [e, NV], f16, kind="ExternalInput")
    wohi_d = nc.dram_tensor("wohi", [NV, e], f16, kind="ExternalInput")
    bqk_d = nc.dram_tensor("bqk", [1, NQK], f16, kind="ExternalInput")
    bv_d = nc.dram_tensor("bv", [1, NV], f16, kind="ExternalInput")
    out_d = nc.dram_tensor("out", [s, e], f32, kind="ExternalOutput")
    if salt:
        nc.dram_tensor(f"salt_{salt}", [1, 1], f32)  # cache-bust marker

    with tile.TileContext(nc) as tc, ExitStack() as ctx:
        const = ctx.enter_context(tc.tile_pool(name="const", bufs=1))
        pp = ctx.enter_context(tc.tile_pool(name="pp", bufs=4, space="PSUM"))
        avp = ctx.enter_context(tc.tile_pool(name="avp", bufs=1, space="PSUM"))
        # T-prefetch pool reserved OUTSIDE the x region so the T stream can
        # warm up during phase 1.
        tp = ctx.enter_context(tc.tile_pool(name="tp", bufs=tp_bufs))

        loop_cm = tc.For_i(0, repeat, 1) if repeat else nullcontext()
        ctx.enter_context(loop_cm)

        # --- persistent tiles ---
        wqk_hi = const.tile([P, NEC, NQK], f16, tag="wqk_hi")
        wqk_lo = const.tile([P, NEC, NQK], f16, tag="wqk_lo")
        wv_hi = const.tile([P, NEC, NV], f16, tag="wv_hi")
        wo_sb = const.tile([P, NDC, e], f16, tag="wo_sb")
        bqk_sb = const.tile([1, NQK], f16, tag="bqk_sb")
        bv_sb = const.tile([1, NV], f16, tag="bv_sb")
        ones = const.tile([1, NB], f16, tag="ones")
        # per-head stores: q duplicated hi/lo ([qhi;qhi] on partitions),
        # k stacked hi over lo ([khi;klo]) for the 2-pass exact product.
        qhi = [const.tile([2 * d, s], f16, tag=f"qhi{h}", name=f"qhi{h}") for h in range(hl)]
        qlo = [const.tile([2 * d, s], f16, tag=f"qlo{h}", name=f"qlo{h}") for h in range(hl)]
        kst = [const.tile([2 * d, s], f16, tag=f"kst{h}", name=f"kst{h}") for h in range(hl)]
        v_sb = const.tile([P, NT, NV], f16, tag="v_sb")
        avT = const.tile([P, NDC, s], f16, tag="avT")

        nc.vector.memset(ones, 1.0)

        # --- phase 1 in an x-scoped pool: its 64KB/partition is recycled for
        # the phase-2 buffers once the projections are done. ---
        with tc.tile_pool(name="xp", bufs=1) as xp:
            x_hi = xp.tile([P, NEC, s], f16, tag="x_hi")
            x_lo = xp.tile([P, NEC, s], f16, tag="x_lo")
            nc.sync.dma_start(x_hi, xhi_d.rearrange("(c p) s -> p c s", p=P))
            nc.sync.dma_start(wv_hi, wvhi_d.rearrange("(c p) n -> p c n", p=P))
            nc.sync.dma_start(bv_sb, bv_d[:, :])
            nc.sync.dma_start(wqk_hi, wqkhi_d.rearrange("(c p) n -> p c n", p=P))
            nc.sync.dma_start(wqk_lo, wqklo_d.rearrange("(c p) n -> p c n", p=P))
            nc.sync.dma_start(x_lo, xlo_d.rearrange("(c p) s -> p c s", p=P))
            nc.sync.dma_start(bqk_sb, bqk_d[:, :])
            nc.sync.dma_start(wo_sb, wohi_d.rearrange("(c p) n -> p c n", p=P))

            # v projection (1-pass fp16)
            for ti in range(NT):
                ps = pp.tile([P, NV], f32, tag="mm")
                nc.tensor.matmul(
                    ps, ones[0:1, 0:P], bv_sb[0:1, :], start=True, stop=False
                )
                for ec in range(NEC):
                    nc.tensor.matmul(
                        ps,
                        x_hi[:, ec, ti * P:(ti + 1) * P],
                        wv_hi[:, ec, :],
                        start=False,
                        stop=(ec == NEC - 1),
                    )
                nc.scalar.activation(v_sb[:, ti, :], ps, Copy)

            # q/k projection (3-pass fp16 hi/lo)
            passes = [(wqk_hi, x_hi), (wqk_hi, x_lo), (wqk_lo, x_hi)]
            for mi in range(MQK):
                for nb in range(NQB):
                    ps = pp.tile([P, NB], f32, tag="mm")
                    nc.tensor.matmul(
                        ps, bqk_sb[0:1, mi * P:(mi + 1) * P], ones[0:1, 0:NB],
                        start=True, stop=False,
                    )
                    for ec in range(NEC):
                        for pi, (wt, xt) in enumerate(passes):
                            nc.tensor.matmul(
                                ps,
                                wt[:, ec, mi * P:(mi + 1) * P],
                                xt[:, ec, nb * NB:(nb + 1) * NB],
                                start=False,
                                stop=(ec == NEC - 1 and pi == len(passes) - 1),
                            )
                    tsl = slice(nb * NB, (nb + 1) * NB)
                    for sub in range(P // d):
                        r = mi * P + sub * d
                        src = ps[sub * d:(sub + 1) * d, :]
                        if r < hl * d:
                            h = r // d
                            nc.scalar.activation(qhi[h][0:d, tsl], src, Copy)
                            nc.scalar.activation(qhi[h][d:2 * d, tsl], src, Copy)
                            nc.vector.tensor_tensor(
                                out=qlo[h][0:d, tsl], in0=src,
                                in1=qhi[h][0:d, tsl], op=AluOpType.subtract,
                            )
                            nc.scalar.activation(
                                qlo[h][d:2 * d, tsl], qlo[h][0:d, tsl], Copy
                            )
                        else:
                            h = (r - hl * d) // d
                            nc.scalar.activation(kst[h][0:d, tsl], src, Copy)
                            nc.vector.tensor_tensor(
                                out=kst[h][d:2 * d, tsl], in0=src,
                                in1=kst[h][0:d, tsl], op=AluOpType.subtract,
                            )

        # --- phase-2 pools (recycle the x region) ---
        scbp = ctx.enter_context(tc.tile_pool(name="scbp", bufs=scb_bufs))
        sp = ctx.enter_context(tc.tile_pool(name="sp", bufs=sm_bufs))
        op = ctx.enter_context(tc.tile_pool(name="op", bufs=2))

        tdummy = samdummy = None
        if skip_tdma:
            tdummy = const.tile([P, s], f32, tag="tdummy")
            nc.vector.memset(tdummy, 0.5)
        if skip_cmpav:
            samdummy = const.tile([P, s], f8, tag="samdummy")
            nc.vector.memset(avT, 0.0)

        # --- phase 2: per head: scores^T -> evict -> sample(fp8) -> attn^T @ v
        def emit_head(h):
            av = None
            if not skip_cmpav:
                av = avp.tile([d, s], f32, tag="av", name=f"av{h}")

            def emit_av(kt, sams):
                for qb, sm in sams:
                    qc = qb * SC_N
                    nc.tensor.matmul(
                        av[0:d, qc:qc + SC_N],
                        v_sb[:, kt, h * d:(h + 1) * d],
                        sm[:, :],
                        start=(kt == 0),
                        stop=(kt == NT - 1),
                    )

            pend = deque()
            for kt in range(NT):
                if not skip_tdma:
                    tt = tp.tile([P, s], f32, tag="tt")
                    nc.sync.dma_start(tt, t_d[h, kt * P:(kt + 1) * P, :])
                else:
                    tt = tdummy
                kl = kst[h][:, kt * P:(kt + 1) * P]
                evs = []
                for qb in range(NSC):
                    sc = pp.tile([P, SC_N], f32, tag="mm")
                    qc = qb * SC_N
                    nc.tensor.matmul(
                        sc, kl, qhi[h][:, qc:qc + SC_N], start=True,
                        stop=skip_lo,
                    )
                    if not skip_lo:
                        nc.tensor.matmul(
                            sc, kl, qlo[h][:, qc:qc + SC_N], start=False, stop=True
                        )
                    scb = scbp.tile([P, SC_N], f32, tag="scb")
                    nc.scalar.activation(scb, sc, Copy)
                    evs.append((qb, scb))
                while len(pend) >= max(1, lag):
                    emit_av(*pend.popleft())
                if skip_cmpav:
                    continue
                sams = []
                for qb, scb in evs:
                    sm = sp.tile([P, SC_N], f8, tag="sm")
                    nc.vector.tensor_tensor(
                        out=sm, in0=tt[:, qb * SC_N:(qb + 1) * SC_N], in1=scb,
                        op=AluOpType.is_lt,
                    )
                    sams.append((qb, sm))
                pend.append((kt, sams))
            while pend:
                emit_av(*pend.popleft())

            if not skip_cmpav:
                dc = (h * d) // P
                r0 = (h * d) % P
                nc.scalar.activation(avT[r0:r0 + d, dc, :], av[0:d, :], Copy)

        for h in range(hl):
            emit_head(h)

        # --- phase 3: out projection (partial; host sums across head groups) ---
        NEB = e // NBE
        for ti in range(NT):
            ob = op.tile([P, NEB, NBE], f32, tag="ob")
            for eb in range(NEB):
                po = pp.tile([P, NBE], f32, tag="mm")
                for dcc in range(NDC):
                    nc.tensor.matmul(
                        po,
                        avT[:, dcc, ti * P:(ti + 1) * P],
                        wo_sb[:, dcc, eb * NBE:(eb + 1) * NBE],
                        start=(dcc == 0),
                        stop=(dcc == NDC - 1),
                    )
                nc.scalar.activation(ob[:, eb, :], po, Copy)
            nc.sync.dma_start(
                out_d.rearrange("s (c n) -> s c n", n=NBE)[
                    ti * P:(ti + 1) * P, :, :
                ],
                ob,
            )

    nc.finalize()
    return nc


# ----------------------------------------------------------------------------
# host side
# ----------------------------------------------------------------------------

def _build_t_shards():
    """T[c, h_local, kk, q] = 8*logit(U[b, h, q, kk]) for core c=(b, h_grp).

    U must be bit-identical to what jax.random.bernoulli(key(42), probs)
    compares against, i.e. jax.random.uniform(key(42), (B,H,S,S), f32) under
    this environment's PRNG impl/backend. Uniform+transpose run through jax;
    logit runs on host in f32 (monotone map, ~1ulp, flips are negligible).
    """
    cache = "/root/problem/t_shards_v2.npy"
    if os.path.exists(cache):
        try:
            t = np.load(cache, mmap_mode=None)
            if t.shape == (NCORES, HL, S, S):
                return np.ascontiguousarray(t)
        except Exception:
            pass

    import jax
    import jax.numpy as jnp

    # IMPORTANT: this exact standalone call produces the same bits as the
    # uniform inside jax.random.bernoulli's compiled program (verified
    # 0/134M mismatch). rbg bits are compilation-dependent, so do NOT fuse
    # anything else (transpose/logit) into this computation.
    u = np.asarray(
        jax.random.uniform(jax.random.key(42), (B, H, S, S), jnp.float32)
    )

    t = np.empty((NCORES, HL, S, S), np.float32)
    one = np.float32(1.0)
    eight = np.float32(8.0)
    gpb = NCORES // B
    with np.errstate(divide="ignore"):
        for b in range(B):
            for h in range(H):
                blk = np.ascontiguousarray(u[b, h].T)  # [kk, q]
                np.divide(blk, one - blk, out=blk)
                np.log(blk, out=blk)
                np.multiply(blk, eight, out=blk)
                t[b * gpb + h // HL, h % HL] = blk
    return t


def _split16(a):
    hi = a.astype(np.float16)
    lo = (a - hi.astype(np.float32)).astype(np.float16)
    return np.ascontiguousarray(hi), np.ascontiguousarray(lo)


def _prep_in_maps(x, qkv_w, qkv_b, out_w, t_shards):
    x = np.asarray(x, np.float32)
    qkv_w = np.asarray(qkv_w, np.float32)
    qkv_b = np.asarray(qkv_b, np.float32)
    out_w = np.asarray(out_w, np.float32)

    xs = []
    for b in range(B):
        xT = np.ascontiguousarray(x[b].T)  # [E, S]
        xs.append(_split16(xT))

    in_maps = []
    for c in range(NCORES):
        b = c // (NCORES // B)
        g = c % (NCORES // B)
        r0 = g * HL * D
        r1 = r0 + HL * D
        wqk = np.concatenate([qkv_w[r0:r1], qkv_w[E + r0:E + r1]], axis=0)
        wqk_hi, wqk_lo = _split16(wqk.T)  # [E, 2*HL*D]
        wv_hi = np.ascontiguousarray(qkv_w[2 * E + r0:2 * E + r1].T).astype(np.float16)
        wo_hi = np.ascontiguousarray(out_w[:, r0:r1].T).astype(np.float16)
        bqk = np.concatenate([qkv_b[r0:r1], qkv_b[E + r0:E + r1]])
        bv = qkv_b[2 * E + r0:2 * E + r1]
        xhi, xlo = xs[b]
        in_maps.append({
            "t": t_shards[c],
            "xhi": xhi, "xlo": xlo,
            "wqkhi": wqk_hi, "wqklo": wqk_lo,
            "wvhi": wv_hi, "wohi": wo_hi,
            "bqk": bqk.astype(np.float16).reshape(1, -1),
            "bv": bv.astype(np.float16).reshape(1, -1),
        })
    return in_maps


def kernel(**inputs):
    from concourse.bass_utils import run_bass_kernel_spmd

    x = np.asarray(inputs["x"], np.float32)
    qkv_w = np.asarray(inputs["qkv_w"], np.float32)
    qkv_b = np.asarray(inputs["qkv_b"], np.float32)
    out_w = np.asarray(inputs["out_w"], np.float32)
    out_b = np.asarray(inputs["out_b"], np.float32)

    if "t" not in _STATE:
        _STATE["t"] = _build_t_shards()
    if "nc" not in _STATE:
        _STATE["nc"] = build_program()

    in_maps = _prep_in_maps(x, qkv_w, qkv_b, out_w, _STATE["t"])
    kw = _STATE.get("run_kwargs", {})
    res = run_bass_kernel_spmd(_STATE["nc"], in_maps, list(range(NCORES)), **kw)
    _STATE["last_results"] = res

    gpb = NCORES // B
    out = np.empty((B, S, E), np.float32)
    for b in range(B):
        acc = res.results[b * gpb]["out"].astype(np.float32)
        for g in range(1, gpb):
            acc = acc + res.results[b * gpb + g]["out"]
        out[b] = acc + out_b[None, :]
    return out
